# revision 1
# baseline (speedup 1.0000x reference)
"""Causal attention (B=4, N=2048, D=1024) on 8 Trainium2 NeuronCores.

Sharding: core 2b+p handles batch b with query tiles {p, p+2, ..., p+14}
(128-row tiles, parity-interleaved).  Every core runs the same program:
8 query slots with key-tile limits (2, 4, ..., 16) — an exactly balanced
causal split.  Per-core masks are passed as input data so the program is
uniform across cores (SPMD).

All matmuls run in float32r (TF32-like, full PE rate at N>=256); fp32
arrays are fed bit-identically into float32r DRAM params (HW rounds at
the PE input).  x is pre-transposed on the host into d-major tile layout
so no on-chip transposes are needed for the projections.

Schedule: Q^T is computed first and spilled to DRAM; then keys are
processed in two halves (V + K^T into SBUF-resident tiles), with
attention slots 0-3 placed between the halves so the scheduler can
overlap early attention with the second half's projections.  Softmax is
single-pass over the full key row (<= 4 PSUM banks) with exp + row-sum
fused on the scalar engine.
"""
import sys

sys.path.insert(0, "/opt/trn_rl_repo")

from contextlib import ExitStack

import numpy as np

import concourse.bass as bass
import concourse.mybir as mybir
import concourse.tile as tile
from concourse import bacc
from concourse.bass_utils import run_bass_kernel_spmd
from concourse.masks import make_identity

B, N, D = 4, 2048, 1024
N_CORES = 8
N_SLOTS = 8          # query tiles per core
N_KTILES = 16        # 128-key tiles per batch
SCALE = 1.0 / 32.0   # 1/sqrt(D)
NEG = -1.0e9

F32 = mybir.dt.float32
F32R = mybir.dt.float32r

_NC_CACHE = {}
TRACE = False
LAST_EXEC_NS = None


def _build_nc():
    nc = bacc.Bacc(None, target_bir_lowering=False, debug=False)

    # x pre-transposed on host: [tile, partition(d%128), dchunk, token]
    x_t = nc.declare_dram_parameter("x_t", [N_KTILES, 128, 8, 128], F32R, isOutput=False)
    x_qt = nc.declare_dram_parameter("x_qt", [N_SLOTS, 128, 8, 128], F32R, isOutput=False)
    # weights host-rearranged: wq/wk [echunk, p(d%128), dchunk, ecol]; wv [eh, p, dchunk, ecol]
    wq = nc.declare_dram_parameter("wq", [8, 128, 8, 128], F32R, isOutput=False)
    wk = nc.declare_dram_parameter("wk", [8, 128, 8, 128], F32R, isOutput=False)
    wv = nc.declare_dram_parameter("wv", [2, 128, 8, 512], F32R, isOutput=False)
    mask_in = nc.declare_dram_parameter("mask", [128, 256], F32, isOutput=False)
    out_q = nc.declare_dram_parameter("out_q", [N_SLOTS, 128, D], F32, isOutput=True)

    # DRAM scratch: Q^T per-slot-contiguous, V spill for key tiles 13..15
    qt_spill = nc.dram_tensor("qt_spill", [N_SLOTS, 128, 8, 128], F32R, kind="Internal")
    v_spill = nc.dram_tensor("v_spill", [2, 128, D], F32R, kind="Internal")

    with tile.TileContext(nc) as tc, ExitStack() as top:
        consts = top.enter_context(tc.tile_pool(name="consts", bufs=1))
        kt_pool = top.enter_context(tc.tile_pool(name="ktp", bufs=1))
        v_pool = top.enter_context(tc.tile_pool(name="vp", bufs=1))
        qt_pool2 = top.enter_context(tc.tile_pool(name="qtl", bufs=2))

        ident_f = consts.tile([128, 128], F32)
        make_identity(nc, ident_f)
        ident = consts.tile([128, 128], F32R)
        nc.vector.tensor_copy(ident, ident_f)
        mask_sb = consts.tile([128, 256], F32)
        nc.sync.dma_start(out=mask_sb, in_=mask_in[:, :])

        KT = kt_pool.tile([128, 8, N], F32R)      # [p(e%128), echunk, key]
        V = v_pool.tile([128, 14, D], F32R)

        with ExitStack() as ph12:
            xt_pool = ph12.enter_context(tc.tile_pool(name="xtp", bufs=1))
            wv_pool = ph12.enter_context(tc.tile_pool(name="wvp", bufs=2))
            we_pool = ph12.enter_context(tc.tile_pool(name="wep", bufs=2))
            qst_pool = ph12.enter_context(tc.tile_pool(name="qst", bufs=1))
            ps_mm = ph12.enter_context(tc.tile_pool(name="ps_mm", bufs=8, space="PSUM"))

            def project_keys(kh):
                """V and K^T for key tiles kh*8 .. kh*8+7."""
                xT = xt_pool.tile([128, 8, 8, 128], F32R, tag="xT", name=f"xk{kh}")
                for lt in range(8):
                    t = kh * 8 + lt
                    nc.gpsimd.dma_start(out=xT[:, lt, :, :], in_=x_t[t][:, :, :])
                for eh in range(2):
                    wv_sb = wv_pool.tile([128, 8, 512], F32R, tag="wv", name=f"wv{kh}_{eh}")
                    for h2 in range(2):
                        nc.scalar.dma_start(
                            out=wv_sb[:, h2 * 4:(h2 + 1) * 4, :],
                            in_=wv[eh][:, h2 * 4:(h2 + 1) * 4, :],
                        )
                    for lt in range(8):
                        t = kh * 8 + lt
                        vps = ps_mm.tile([128, 512], F32, tag="mm", name=f"v{kh}_{eh}_{lt}")
                        for c in range(8):
                            nc.tensor.matmul(
                                vps, xT[:, lt, c, :], wv_sb[:, c, :],
                                start=(c == 0), stop=(c == 7),
                            )
                        if t < 14:
                            nc.vector.tensor_copy(V[:, t, eh * 512:(eh + 1) * 512], vps)
                        else:
                            vst = qst_pool.tile([128, 512], F32R, tag="qs", name=f"vs{t}_{eh}")
                            nc.vector.tensor_copy(vst, vps)
                            nc.sync.dma_start(
                                out=v_spill[t - 14][:, eh * 512:(eh + 1) * 512], in_=vst
                            )
                for e in range(8):
                    wk_sb = we_pool.tile([128, 8, 128], F32R, tag="we", name=f"wk{kh}_{e}")
                    nc.scalar.dma_start(out=wk_sb, in_=wk[e][:, :, :])
                    kps = [ps_mm.tile([128, 512], F32, tag="mm", name=f"k{kh}_{e}_{g}")
                           for g in range(2)]
                    for c in range(8):
                        for kg in range(2):
                            nc.tensor.matmul(
                                kps[kg], wk_sb[:, c, :], xT[:, kg * 4:(kg + 1) * 4, c, :],
                                start=(c == 0), stop=(c == 7),
                            )
                    for kg in range(2):
                        nc.vector.tensor_copy(
                            KT[:, e, (kh * 2 + kg) * 512:(kh * 2 + kg + 1) * 512], kps[kg]
                        )

            def project_queries():
                xT = xt_pool.tile([128, 8, 8, 128], F32R, tag="xT", name="xq")
                for s in range(N_SLOTS):
                    nc.gpsimd.dma_start(out=xT[:, s, :, :], in_=x_qt[s][:, :, :])
                for e in range(8):
                    wq_sb = we_pool.tile([128, 8, 128], F32R, tag="we", name=f"wq{e}")
                    nc.scalar.dma_start(out=wq_sb, in_=wq[e][:, :, :])
                    qps = [ps_mm.tile([128, 512], F32, tag="mm", name=f"q{e}_{g}")
                           for g in range(2)]
                    for c in range(8):
                        for qg in range(2):
                            nc.tensor.matmul(
                                qps[qg], wq_sb[:, c, :], xT[:, qg * 4:(qg + 1) * 4, c, :],
                                start=(c == 0), stop=(c == 7),
                            )
                    qstage = qst_pool.tile([128, 1024], F32R, tag="qs", name=f"qs{e}")
                    for qg in range(2):
                        nc.vector.tensor_copy(qstage[:, qg * 512:(qg + 1) * 512], qps[qg])
                    nc.sync.dma_start(
                        out=qt_spill[:, :, e, :].rearrange("s p q -> p s q"),
                        in_=qstage.rearrange("p (s q) -> p s q", s=8),
                    )

            project_keys(0)
            project_queries()  # qt spill roundtrip + kh1 x loads hide here
            project_keys(1)

        # ---- attention slots 0-7, software-pipelined AV ----
        with ExitStack() as ph3:
            ps_tr = ph3.enter_context(tc.tile_pool(name="ps_tr", bufs=2, space="PSUM"))
            ps_o = ph3.enter_context(tc.tile_pool(name="ps_o", bufs=1, space="PSUM"))
            p_hi = ph3.enter_context(tc.tile_pool(name="phi", bufs=2))
            pt_pool = ph3.enter_context(tc.tile_pool(name="ptp", bufs=2))
            sc_pool = ph3.enter_context(tc.tile_pool(name="scp", bufs=2))
            outp = ph3.enter_context(tc.tile_pool(name="outp", bufs=2))
            vh_pool = ph3.enter_context(tc.tile_pool(name="vhp", bufs=1))
            v_hi = []

            def emit_av(i, L, P_sb, recip):
                O_ps = ps_o.tile([128, D], F32, tag="O", name=f"O{i}")
                for kt in range(L):
                    ptps = ps_tr.tile([128, 128], F32R, tag="tr", name=f"tp{i}_{kt}")
                    nc.tensor.transpose(ptps, P_sb[:, kt * 128:(kt + 1) * 128], ident)
                    pt_sb = pt_pool.tile([128, 128], F32R, tag="pts", name=f"pt{i}_{kt}")
                    nc.vector.tensor_copy(pt_sb, ptps)
                    vsrc = V[:, kt, :] if kt < 14 else v_hi[kt - 14]
                    for h in range(2):
                        nc.tensor.matmul(
                            O_ps[:, h * 512:(h + 1) * 512], pt_sb,
                            vsrc[:, h * 512:(h + 1) * 512],
                            start=(kt == 0), stop=(kt == L - 1),
                        )
                out_sb = outp.tile([128, D], F32, tag="osb", name=f"ou{i}")
                nc.vector.tensor_scalar_mul(out_sb, O_ps, recip)
                nc.sync.dma_start(out=out_q[i][:, :], in_=out_sb)

            def do_slot(i, ps_pool, s_width, prev):
                L = 2 * (i + 1)
                qt_sb = qt_pool2.tile([128, 8, 128], F32R, tag="qt", name=f"qt{i}")
                nc.gpsimd.dma_start(out=qt_sb, in_=qt_spill[i][:, :, :])
                S_ps = ps_pool.tile([128, s_width], F32, tag="S", name=f"S{i}")
                ngroups = (L * 128 + 511) // 512
                for e in range(8):
                    for kg in range(ngroups):
                        w = min(512, L * 128 - kg * 512)
                        nc.tensor.matmul(
                            S_ps[:, kg * 512: kg * 512 + w],
                            qt_sb[:, e, :],
                            KT[:, e, kg * 512: kg * 512 + w],
                            start=(e == 0), stop=(e == 7),
                        )
                # scores/32 are bounded (|s|/32 <~ 11) -> exp without max-subtraction
                nc.vector.tensor_add(
                    S_ps[:, (L - 2) * 128: L * 128],
                    S_ps[:, (L - 2) * 128: L * 128],
                    mask_sb,
                )
                P_sb = p_hi.tile([128, N], F32R, tag="P", name=f"P{i}")
                stats = sc_pool.tile([128, 4], F32, tag="stats", name=f"st{i}")
                rowsum = stats[:, 2:3]
                nc.scalar.activation(
                    P_sb[:, : L * 128], S_ps[:, : L * 128],
                    mybir.ActivationFunctionType.Exp,
                    bias=0.0, scale=SCALE, accum_out=rowsum,
                )
                recip = stats[:, 3:4]
                nc.vector.reciprocal(recip, rowsum)
                if prev is not None:
                    emit_av(*prev)
                return (i, L, P_sb, recip)

            prev = None
            with tc.tile_pool(name="ps_sA", bufs=2, space="PSUM") as ps_sA:
                for i in range(4):
                    prev = do_slot(i, ps_sA, 1024, prev)
            with tc.tile_pool(name="ps_sB", bufs=1, space="PSUM") as ps_sB:
                for i in range(4, 6):
                    prev = do_slot(i, ps_sB, 2048, prev)
                for j in range(2):
                    vh = vh_pool.tile([128, D], F32R, tag=f"vh{j}", name=f"vh{j}")
                    nc.sync.dma_start(out=vh, in_=v_spill[j][:, :])
                    v_hi.append(vh)
                for i in range(6, N_SLOTS):
                    prev = do_slot(i, ps_sB, 2048, prev)
                emit_av(*prev)

    nc.compile()
    return nc


def _masks():
    q = np.arange(128)[:, None]
    k = np.arange(128)[None, :]
    tril_add = np.where(k <= q, 0.0, NEG).astype(np.float32)
    m0 = np.concatenate([tril_add, np.full((128, 128), NEG, np.float32)], axis=1)
    m1 = np.concatenate([np.zeros((128, 128), np.float32), tril_add], axis=1)
    return m0, m1


def kernel(x, Wq, Wk, Wv):
    global LAST_EXEC_NS
    x = np.ascontiguousarray(np.asarray(x, dtype=np.float32))
    Wq = np.ascontiguousarray(np.asarray(Wq, dtype=np.float32))
    Wk = np.ascontiguousarray(np.asarray(Wk, dtype=np.float32))
    Wv = np.ascontiguousarray(np.asarray(Wv, dtype=np.float32))

    if "nc" not in _NC_CACHE:
        _NC_CACHE["nc"] = _build_nc()
    nc = _NC_CACHE["nc"]

    # host pre-transpose: x[b] (N, D) -> (tile, p=d%128, dchunk, token)
    # element (t, p, c, q) = x[b, t*128+q, c*128+p]
    xt_all = np.ascontiguousarray(
        x.reshape(B, N_KTILES, 128, 8, 128).transpose(0, 1, 4, 3, 2)
    )  # [B, tile, p, c, q]

    # weights host-rearranged to give contiguous per-partition DMA runs
    wq_r = np.ascontiguousarray(Wq.reshape(8, 128, 8, 128).transpose(2, 1, 0, 3))
    wk_r = np.ascontiguousarray(Wk.reshape(8, 128, 8, 128).transpose(2, 1, 0, 3))
    wv_r = np.ascontiguousarray(Wv.reshape(8, 128, 2, 512).transpose(2, 1, 0, 3))

    m0, m1 = _masks()
    in_maps = []
    for c in range(N_CORES):
        b, par = divmod(c, 2)
        in_maps.append({
            "x_t": xt_all[b],
            "x_qt": np.ascontiguousarray(xt_all[b, par::2]),
            "wq": wq_r, "wk": wk_r, "wv": wv_r,
            "mask": m1 if par else m0,
        })

    res = run_bass_kernel_spmd(nc, in_maps, list(range(N_CORES)), trace=TRACE)
    LAST_EXEC_NS = res.exec_time_ns

    out = np.empty((B, N, D), dtype=np.float32)
    for c in range(N_CORES):
        b, par = divmod(c, 2)
        oq = res.results[c]["out_q"]
        for i in range(N_SLOTS):
            g = 2 * i + par
            out[b, g * 128:(g + 1) * 128, :] = oq[i]
    return out



# revision 3
# speedup vs baseline: 1.2497x; 1.2497x over previous
"""Causal attention (B=4, N=2048, D=1024) on 8 Trainium2 NeuronCores.

v2 design (vs baseline):
  * All on-chip tensors bf16 (tolerance 2e-2; measured numpy pipeline err
    ~4e-3).  Halves DMA bytes and SBUF so K^T, V and Q^T stay fully
    SBUF-resident -- no DRAM spill roundtrips.
  * Scores computed TRANSPOSED (S^T[k,q] per key tile): the exp'd P^T is
    directly the stationary operand of the AV matmul, eliminating all PE
    transposes and the exp->transpose->copy->AV serial chain.  Row sums
    come from a 1-wide matmul against a ones vector that reuses the AV
    stationary (P^T) already loaded in the array.
  * Core 2b+s handles batch b; s=0 takes query tiles {0,2,4,6, 9,11,13,15},
    s=1 takes {1,3,5,7, 8,10,12,14} -- both sum to 68 causal key-tile pairs.
    The program is SPMD-uniform with key limits (2,4,..,16); the per-core
    diagonal/full masks are passed as input data ([128,512] = group1 pair +
    group2 pair of 128-col blocks).
  * Slot groups of 4 give 512-wide moving operands for S^T; widths taper
    (512/384/256/128) following the causal staircase.
  * x double-buffered across the two key-tile phases; weights loaded once.
"""
import sys

sys.path.insert(0, "/opt/trn_rl_repo")

from contextlib import ExitStack

import numpy as np
import ml_dtypes

import concourse.bass as bass
import concourse.mybir as mybir
import concourse.tile as tile
from concourse import bacc
from concourse.bass_utils import run_bass_kernel_spmd

B, N, D = 4, 2048, 1024
N_CORES = 8
N_SLOTS = 8
N_KTILES = 16
SCALE = 1.0 / 32.0   # 1/sqrt(D)
NEG = -1.0e9

F32 = mybir.dt.float32
BF16 = mybir.dt.bfloat16
BF = ml_dtypes.bfloat16

# query-tile sets per parity slot s (ascending); both have sum(g+1) == 68
QSETS = [
    [0, 2, 4, 6, 9, 11, 13, 15],
    [1, 3, 5, 7, 8, 10, 12, 14],
]
# uniform program limits per slot (key tiles 0..L-1 computed)
LIMITS = [2, 4, 6, 8, 10, 12, 14, 16]

_NC_CACHE = {}
TRACE = False
LAST_EXEC_NS = None


def _build_nc():
    nc = bacc.Bacc(None, target_bir_lowering=False, debug=False)

    # x tile layout: [tile, p=d%128, dchunk, token]
    x_t = nc.declare_dram_parameter("x_t", [N_KTILES, 128, 8, 128], BF16, isOutput=False)
    x_qt = nc.declare_dram_parameter("x_qt", [N_SLOTS, 128, 8, 128], BF16, isOutput=False)
    # weights: [p=d%128, dchunk, ecol]
    wq = nc.declare_dram_parameter("wq", [128, 8, 1024], BF16, isOutput=False)
    wk = nc.declare_dram_parameter("wk", [128, 8, 1024], BF16, isOutput=False)
    wv = nc.declare_dram_parameter("wv", [128, 8, 1024], BF16, isOutput=False)
    mask_in = nc.declare_dram_parameter("mask", [128, 512], F32, isOutput=False)
    out_q = nc.declare_dram_parameter("out_q", [N_SLOTS, 128, D], F32, isOutput=True)

    with tile.TileContext(nc) as tc, ExitStack() as top:
        consts = top.enter_context(tc.tile_pool(name="consts", bufs=1))
        kt_pool = top.enter_context(tc.tile_pool(name="ktp", bufs=1))
        v_pool = top.enter_context(tc.tile_pool(name="vp", bufs=1))
        qt_pool = top.enter_context(tc.tile_pool(name="qtp", bufs=1))

        mask_sb = consts.tile([128, 512], F32)
        nc.sync.dma_start(out=mask_sb, in_=mask_in[:, :])
        ones = consts.tile([128, 8], BF16)
        nc.vector.memset(ones, 1.0)

        KT = kt_pool.tile([128, 8, N], BF16)     # [p=e%128, echunk, key]
        V = v_pool.tile([128, N_KTILES, D], BF16)  # [p=key%128, ktile, e]
        QT = qt_pool.tile([128, 8, 1024], BF16)  # [p=e%128, echunk, qcol]

        with ExitStack() as ph12:
            xt_pool = ph12.enter_context(tc.tile_pool(name="xtp", bufs=2))
            qxt_pool = ph12.enter_context(tc.tile_pool(name="qxt", bufs=1))
            w_pool = ph12.enter_context(tc.tile_pool(name="wp", bufs=1))
            ps_mm = ph12.enter_context(tc.tile_pool(name="ps_mm", bufs=8, space="PSUM"))

            wv_sb = w_pool.tile([128, 8, 1024], BF16, tag="wv")
            for eh in range(2):
                nc.scalar.dma_start(
                    out=wv_sb[:, :, eh * 512:(eh + 1) * 512],
                    in_=wv[:, :, eh * 512:(eh + 1) * 512],
                )
            wk_sb = w_pool.tile([128, 8, 1024], BF16, tag="wk")
            nc.scalar.dma_start(out=wk_sb, in_=wk[:, :, :])
            wq_sb = w_pool.tile([128, 8, 1024], BF16, tag="wq")
            nc.scalar.dma_start(out=wq_sb, in_=wq[:, :, :])

            QXT = qxt_pool.tile([128, 8, 8, 128], BF16, tag="qx")
            for s in range(N_SLOTS):
                nc.sync.dma_start(out=QXT[:, s, :, :], in_=x_qt[s][:, :, :])

            def proj_phase(kh):
                xT = xt_pool.tile([128, 8, 8, 128], BF16, tag="xT", name=f"x{kh}")
                for lt in range(8):
                    nc.gpsimd.dma_start(out=xT[:, lt, :, :], in_=x_t[kh * 8 + lt][:, :, :])
                # V projection: stationary x chunk shared across both e-halves
                for lt in range(8):
                    t = kh * 8 + lt
                    vps = [ps_mm.tile([128, 512], F32, tag="mm", name=f"v{t}_{eh}")
                           for eh in range(2)]
                    for c in range(8):
                        for eh in range(2):
                            nc.tensor.matmul(
                                vps[eh], xT[:, lt, c, :],
                                wv_sb[:, c, eh * 512:(eh + 1) * 512],
                                start=(c == 0), stop=(c == 7),
                            )
                    for eh in range(2):
                        nc.vector.tensor_copy(V[:, t, eh * 512:(eh + 1) * 512], vps[eh])
                # K^T projection: stationary W chunk shared across both key groups
                for e in range(8):
                    kps = [ps_mm.tile([128, 512], F32, tag="mm", name=f"k{kh}_{e}_{g}")
                           for g in range(2)]
                    for c in range(8):
                        for kg in range(2):
                            nc.tensor.matmul(
                                kps[kg], wk_sb[:, c, e * 128:(e + 1) * 128],
                                xT[:, kg * 4:(kg + 1) * 4, c, :],
                                start=(c == 0), stop=(c == 7),
                            )
                    for kg in range(2):
                        key0 = (kh * 8 + kg * 4) * 128
                        nc.vector.tensor_copy(KT[:, e, key0:key0 + 512], kps[kg])

            def proj_queries():
                # both slot groups; stationary W chunk shared across groups
                for e in range(8):
                    qps = [ps_mm.tile([128, 512], F32, tag="mm", name=f"q{e}_{g}")
                           for g in range(2)]
                    for c in range(8):
                        for g in range(2):
                            nc.tensor.matmul(
                                qps[g], wq_sb[:, c, e * 128:(e + 1) * 128],
                                QXT[:, g * 4:(g + 1) * 4, c, :],
                                start=(c == 0), stop=(c == 7),
                            )
                    for g in range(2):
                        nc.vector.tensor_copy(QT[:, e, g * 512:(g + 1) * 512], qps[g])

            proj_phase(0)
            proj_queries()
            proj_phase(1)

        # ---- attention: S^T per key tile, then AV with P^T stationary ----
        with ExitStack() as ph3:
            pt_pool = ph3.enter_context(tc.tile_pool(name="ptp", bufs=1))
            ps_st = ph3.enter_context(tc.tile_pool(name="ps_st", bufs=2, space="PSUM"))
            ps_o = ph3.enter_context(tc.tile_pool(name="ps_o", bufs=2, space="PSUM"))
            ps_rs = ph3.enter_context(tc.tile_pool(name="ps_rs", bufs=2, space="PSUM"))
            sc_pool = ph3.enter_context(tc.tile_pool(name="scp", bufs=2))
            outp = ph3.enter_context(tc.tile_pool(name="outp", bufs=2))

            PTs = [
                pt_pool.tile([128, 8, 512], BF16, tag="pt1", name="PT1"),
                pt_pool.tile([128, 16, 512], BF16, tag="pt2", name="PT2"),
            ]

            def st_phase(g):
                PT = PTs[g]
                Ls = LIMITS[g * 4:(g + 1) * 4]
                for kt in range(Ls[-1]):
                    f = sum(1 for L in Ls if L <= kt)   # first participating slot
                    w = (4 - f) * 128
                    col0 = f * 128
                    sps = ps_st.tile([128, 512], F32, tag="st", name=f"s{g}_{kt}")
                    for c in range(8):
                        nc.tensor.matmul(
                            sps[:, 0:w],
                            KT[:, c, kt * 128:(kt + 1) * 128],
                            QT[:, c, g * 512 + col0: g * 512 + col0 + w],
                            start=(c == 0), stop=(c == 7),
                        )
                    if kt == Ls[f] - 2:
                        nc.vector.tensor_add(
                            sps[:, 0:128], sps[:, 0:128],
                            mask_sb[:, g * 256: g * 256 + 128],
                        )
                    elif kt == Ls[f] - 1:
                        nc.vector.tensor_add(
                            sps[:, 0:128], sps[:, 0:128],
                            mask_sb[:, g * 256 + 128: g * 256 + 256],
                        )
                    nc.scalar.activation(
                        PT[:, kt, col0:col0 + w], sps[:, 0:w],
                        mybir.ActivationFunctionType.Exp,
                        bias=0.0, scale=SCALE,
                    )

            def av_slot(g, j):
                PT = PTs[g]
                slot = g * 4 + j
                L = LIMITS[slot]
                col = j * 128
                O_ps = ps_o.tile([128, D], F32, tag="O", name=f"O{slot}")
                rs_ps = ps_rs.tile([128, 1], F32, tag="rs", name=f"r{slot}")
                for kt in range(L):
                    pt_blk = PT[:, kt, col:col + 128]
                    for h in range(2):
                        nc.tensor.matmul(
                            O_ps[:, h * 512:(h + 1) * 512], pt_blk,
                            V[:, kt, h * 512:(h + 1) * 512],
                            start=(kt == 0), stop=(kt == L - 1),
                        )
                    nc.tensor.matmul(
                        rs_ps, pt_blk, ones[:, 0:1],
                        start=(kt == 0), stop=(kt == L - 1),
                    )
                stats = sc_pool.tile([128, 8], F32, tag="stats", name=f"st{slot}")
                recip = stats[:, 0:1]
                nc.vector.reciprocal(recip, rs_ps)
                out_sb = outp.tile([128, D], F32, tag="osb", name=f"ou{slot}")
                nc.vector.tensor_scalar_mul(out_sb, O_ps, recip)
                nc.sync.dma_start(out=out_q[slot][:, :], in_=out_sb)

            st_phase(0)
            for j in range(4):
                av_slot(0, j)
            st_phase(1)
            for j in range(4):
                av_slot(1, j)

    nc.compile()
    return nc


def _masks():
    k = np.arange(128)[:, None]
    q = np.arange(128)[None, :]
    tril_t = np.where(k <= q, 0.0, NEG).astype(np.float32)  # S^T diag block
    fullneg = np.full((128, 128), NEG, np.float32)
    zeros = np.zeros((128, 128), np.float32)
    m_s0 = np.concatenate([tril_t, fullneg, zeros, tril_t], axis=1)
    m_s1 = np.concatenate([zeros, tril_t, tril_t, fullneg], axis=1)
    return m_s0, m_s1


def kernel(x, Wq, Wk, Wv):
    global LAST_EXEC_NS
    x = np.asarray(x, dtype=np.float32)
    Wq = np.asarray(Wq, dtype=np.float32)
    Wk = np.asarray(Wk, dtype=np.float32)
    Wv = np.asarray(Wv, dtype=np.float32)

    if "nc" not in _NC_CACHE:
        _NC_CACHE["nc"] = _build_nc()
    nc = _NC_CACHE["nc"]

    # host pre-transpose: x[b] (N, D) -> (tile, p=d%128, dchunk, token), bf16
    xt_all = np.ascontiguousarray(
        x.reshape(B, N_KTILES, 128, 8, 128).transpose(0, 1, 4, 3, 2).astype(BF)
    )  # [B, tile, p, c, q]

    # weights -> [p=d%128, dchunk, ecol], bf16
    wq_r = np.ascontiguousarray(Wq.reshape(8, 128, 1024).transpose(1, 0, 2).astype(BF))
    wk_r = np.ascontiguousarray(Wk.reshape(8, 128, 1024).transpose(1, 0, 2).astype(BF))
    wv_r = np.ascontiguousarray(Wv.reshape(8, 128, 1024).transpose(1, 0, 2).astype(BF))

    m_s0, m_s1 = _masks()
    in_maps = []
    for c in range(N_CORES):
        b, s = divmod(c, 2)
        in_maps.append({
            "x_t": xt_all[b],
            "x_qt": np.ascontiguousarray(xt_all[b, QSETS[s]]),
            "wq": wq_r, "wk": wk_r, "wv": wv_r,
            "mask": m_s1 if s else m_s0,
        })

    res = run_bass_kernel_spmd(nc, in_maps, list(range(N_CORES)), trace=TRACE)
    LAST_EXEC_NS = res.exec_time_ns

    out = np.empty((B, N, D), dtype=np.float32)
    for c in range(N_CORES):
        b, s = divmod(c, 2)
        oq = res.results[c]["out_q"]
        for j, g in enumerate(QSETS[s]):
            out[b, g * 128:(g + 1) * 128, :] = oq[j]
    return out


# revision 8
# speedup vs baseline: 1.2964x; 1.0374x over previous
"""Causal attention (B=4, N=2048, D=1024) on 8 Trainium2 NeuronCores.

v2 design (vs baseline):
  * All on-chip tensors bf16 (tolerance 2e-2; measured numpy pipeline err
    ~4e-3).  Halves DMA bytes and SBUF so K^T, V and Q^T stay fully
    SBUF-resident -- no DRAM spill roundtrips.
  * Scores computed TRANSPOSED (S^T[k,q] per key tile): the exp'd P^T is
    directly the stationary operand of the AV matmul, eliminating all PE
    transposes and the exp->transpose->copy->AV serial chain.  Row sums
    come from a 1-wide matmul against a ones vector that reuses the AV
    stationary (P^T) already loaded in the array.
  * Core 2b+s handles batch b; s=0 takes query tiles {0,2,4,6, 9,11,13,15},
    s=1 takes {1,3,5,7, 8,10,12,14} -- both sum to 68 causal key-tile pairs.
    The program is SPMD-uniform with key limits (2,4,..,16); the per-core
    diagonal/full masks are passed as input data ([128,512] = group1 pair +
    group2 pair of 128-col blocks).
  * Slot groups of 4 give 512-wide moving operands for S^T; widths taper
    (512/384/256/128) following the causal staircase.
  * x double-buffered across the two key-tile phases; weights loaded once.
"""
import sys

sys.path.insert(0, "/opt/trn_rl_repo")

from contextlib import ExitStack

import numpy as np
import ml_dtypes

import concourse.bass as bass
import concourse.mybir as mybir
import concourse.tile as tile
from concourse import bacc
from concourse.bass_utils import run_bass_kernel_spmd

B, N, D = 4, 2048, 1024
N_CORES = 8
N_SLOTS = 8
N_KTILES = 16
SCALE = 1.0 / 32.0   # 1/sqrt(D)
NEG = -1.0e9

F32 = mybir.dt.float32
BF16 = mybir.dt.bfloat16
BF = ml_dtypes.bfloat16

# query-tile sets per parity slot s (ascending); both have sum(g+1) == 68
QSETS = [
    [0, 2, 4, 6, 9, 11, 13, 15],
    [1, 3, 5, 7, 8, 10, 12, 14],
]
# uniform program limits per slot (key tiles 0..L-1 computed)
LIMITS = [2, 4, 6, 8, 10, 12, 14, 16]

_NC_CACHE = {}
TRACE = False
LAST_EXEC_NS = None


def _build_nc():
    nc = bacc.Bacc(None, target_bir_lowering=False, debug=False)

    # x tile layout: [tile, p=d%128, dchunk, token]
    x_t = nc.declare_dram_parameter("x_t", [N_KTILES, 128, 8, 128], BF16, isOutput=False)
    x_qt = nc.declare_dram_parameter("x_qt", [N_SLOTS, 128, 8, 128], BF16, isOutput=False)
    # weights: [p=d%128, dchunk, ecol]
    wq = nc.declare_dram_parameter("wq", [128, 8, 1024], BF16, isOutput=False)
    wk = nc.declare_dram_parameter("wk", [128, 8, 1024], BF16, isOutput=False)
    wv = nc.declare_dram_parameter("wv", [128, 8, 1024], BF16, isOutput=False)
    mask_in = nc.declare_dram_parameter("mask", [128, 512], F32, isOutput=False)
    out_q = nc.declare_dram_parameter("out_q", [N_SLOTS, 128, D], F32, isOutput=True)

    with tile.TileContext(nc) as tc, ExitStack() as top:
        consts = top.enter_context(tc.tile_pool(name="consts", bufs=1))
        kt_pool = top.enter_context(tc.tile_pool(name="ktp", bufs=1))
        v_pool = top.enter_context(tc.tile_pool(name="vp", bufs=1))
        qt_pool = top.enter_context(tc.tile_pool(name="qtp", bufs=1))

        ones = consts.tile([128, 8], BF16)
        nc.vector.memset(ones, 1.0)
        mask_sb = consts.tile([128, 512], F32)

        KT = kt_pool.tile([128, 8, N], BF16)     # [p=e%128, echunk, key]
        V = v_pool.tile([128, N_KTILES, D], BF16)  # [p=key%128, ktile, e]
        QT = qt_pool.tile([128, 8, 1024], BF16)  # [p=e%128, echunk, qcol]

        with ExitStack() as ph12:
            xt_pool = ph12.enter_context(tc.tile_pool(name="xtp", bufs=2))
            qxt_pool = ph12.enter_context(tc.tile_pool(name="qxt", bufs=1))
            w_pool = ph12.enter_context(tc.tile_pool(name="wp", bufs=1))
            ps_mm = ph12.enter_context(tc.tile_pool(name="ps_mm", bufs=8, space="PSUM"))

            # one bulk DMA per weight, spread across queues so they stream in
            # parallel (per-queue DMA BW is ~100-180 GB/s, well under core BW)
            wv_sb = w_pool.tile([128, 8, 1024], BF16, tag="wv")
            nc.scalar.dma_start(out=wv_sb, in_=wv[:, :, :])
            wk_sb = w_pool.tile([128, 8, 1024], BF16, tag="wk")
            wq_sb = w_pool.tile([128, 8, 1024], BF16, tag="wq")
            nc.sync.dma_start(out=wq_sb, in_=wq[:, :, :])

            QXT = qxt_pool.tile([128, 8, 8, 128], BF16, tag="qx")
            nc.sync.dma_start(
                out=QXT, in_=x_qt[:].rearrange("s p c q -> p s c q")
            )
            nc.sync.dma_start(out=mask_sb, in_=mask_in[:, :])

            def proj_phase(kh):
                xT = xt_pool.tile([128, 8, 8, 128], BF16, tag="xT", name=f"x{kh}")
                if kh == 0:
                    # per-tile DMAs: first V chain starts after one tile lands
                    for lt in range(8):
                        nc.gpsimd.dma_start(
                            out=xT[:, lt, :, :], in_=x_t[lt][:, :, :]
                        )
                    # wk rides the gpsimd queue behind the kh0 x tiles;
                    # K proj needs it only after the 8 V chains (~25us in)
                    nc.gpsimd.dma_start(out=wk_sb, in_=wk[:, :, :])
                else:
                    # bulk prefetch on the scalar queue (free after wv)
                    nc.scalar.dma_start(
                        out=xT, in_=x_t[8:16].rearrange("t p c q -> p t c q")
                    )
                # V projection: stationary x chunk shared across both e-halves
                for lt in range(8):
                    t = kh * 8 + lt
                    vps = [ps_mm.tile([128, 512], F32, tag="mm", name=f"v{t}_{eh}")
                           for eh in range(2)]
                    for c in range(8):
                        for eh in range(2):
                            nc.tensor.matmul(
                                vps[eh], xT[:, lt, c, :],
                                wv_sb[:, c, eh * 512:(eh + 1) * 512],
                                start=(c == 0), stop=(c == 7),
                            )
                    for eh in range(2):
                        nc.vector.tensor_copy(V[:, t, eh * 512:(eh + 1) * 512], vps[eh])
                # K^T projection: stationary W chunk shared across both key groups
                for e in range(8):
                    kps = [ps_mm.tile([128, 512], F32, tag="mm", name=f"k{kh}_{e}_{g}")
                           for g in range(2)]
                    for c in range(8):
                        for kg in range(2):
                            nc.tensor.matmul(
                                kps[kg], wk_sb[:, c, e * 128:(e + 1) * 128],
                                xT[:, kg * 4:(kg + 1) * 4, c, :],
                                start=(c == 0), stop=(c == 7),
                            )
                    for kg in range(2):
                        key0 = (kh * 8 + kg * 4) * 128
                        nc.vector.tensor_copy(KT[:, e, key0:key0 + 512], kps[kg])

            def proj_queries():
                # both slot groups; stationary W chunk shared across groups
                for e in range(8):
                    qps = [ps_mm.tile([128, 512], F32, tag="mm", name=f"q{e}_{g}")
                           for g in range(2)]
                    for c in range(8):
                        for g in range(2):
                            nc.tensor.matmul(
                                qps[g], wq_sb[:, c, e * 128:(e + 1) * 128],
                                QXT[:, g * 4:(g + 1) * 4, c, :],
                                start=(c == 0), stop=(c == 7),
                            )
                    for g in range(2):
                        nc.vector.tensor_copy(QT[:, e, g * 512:(g + 1) * 512], qps[g])

            proj_phase(0)
            proj_queries()
            proj_phase(1)

        # ---- attention: S^T per key tile, then AV with P^T stationary ----
        with ExitStack() as ph3:
            pt_pool = ph3.enter_context(tc.tile_pool(name="ptp", bufs=1))
            ps_st = ph3.enter_context(tc.tile_pool(name="ps_st", bufs=2, space="PSUM"))
            ps_o = ph3.enter_context(tc.tile_pool(name="ps_o", bufs=2, space="PSUM"))
            ps_rs = ph3.enter_context(tc.tile_pool(name="ps_rs", bufs=2, space="PSUM"))
            sc_pool = ph3.enter_context(tc.tile_pool(name="scp", bufs=2))
            outp = ph3.enter_context(tc.tile_pool(name="outp", bufs=2))

            PTs = [
                pt_pool.tile([128, 8, 512], BF16, tag="pt1", name="PT1"),
                pt_pool.tile([128, 16, 512], BF16, tag="pt2", name="PT2"),
            ]

            def st_phase(g):
                PT = PTs[g]
                Ls = LIMITS[g * 4:(g + 1) * 4]
                for kt in range(Ls[-1]):
                    f = sum(1 for L in Ls if L <= kt)   # first participating slot
                    w = (4 - f) * 128
                    col0 = f * 128
                    sps = ps_st.tile([128, 512], F32, tag="st", name=f"s{g}_{kt}")
                    for c in range(8):
                        nc.tensor.matmul(
                            sps[:, 0:w],
                            KT[:, c, kt * 128:(kt + 1) * 128],
                            QT[:, c, g * 512 + col0: g * 512 + col0 + w],
                            start=(c == 0), stop=(c == 7),
                        )
                    if kt == Ls[f] - 2:
                        nc.vector.tensor_add(
                            sps[:, 0:128], sps[:, 0:128],
                            mask_sb[:, g * 256: g * 256 + 128],
                        )
                    elif kt == Ls[f] - 1:
                        nc.vector.tensor_add(
                            sps[:, 0:128], sps[:, 0:128],
                            mask_sb[:, g * 256 + 128: g * 256 + 256],
                        )
                    nc.scalar.activation(
                        PT[:, kt, col0:col0 + w], sps[:, 0:w],
                        mybir.ActivationFunctionType.Exp,
                        bias=0.0, scale=SCALE,
                    )

            def av_slot(g, j):
                PT = PTs[g]
                slot = g * 4 + j
                L = LIMITS[slot]
                col = j * 128
                O_ps = ps_o.tile([128, D], F32, tag="O", name=f"O{slot}")
                rs_ps = ps_rs.tile([128, 1], F32, tag="rs", name=f"r{slot}")
                for kt in range(L):
                    pt_blk = PT[:, kt, col:col + 128]
                    for h in range(2):
                        nc.tensor.matmul(
                            O_ps[:, h * 512:(h + 1) * 512], pt_blk,
                            V[:, kt, h * 512:(h + 1) * 512],
                            start=(kt == 0), stop=(kt == L - 1),
                        )
                    nc.tensor.matmul(
                        rs_ps, pt_blk, ones[:, 0:1],
                        start=(kt == 0), stop=(kt == L - 1),
                    )
                stats = sc_pool.tile([128, 8], F32, tag="stats", name=f"st{slot}")
                recip = stats[:, 0:1]
                nc.vector.reciprocal(recip, rs_ps)
                out_sb = outp.tile([128, D], F32, tag="osb", name=f"ou{slot}")
                nc.vector.tensor_scalar_mul(out_sb, O_ps, recip)
                # alternate output queues so the final drain is parallel
                eng = nc.sync if slot % 2 == 0 else nc.gpsimd
                eng.dma_start(out=out_q[slot][:, :], in_=out_sb)

            # descending L within each group: the big slots' outputs DMA out
            # early, shrinking the end-of-kernel drain
            st_phase(0)
            for j in (3, 2, 1, 0):
                av_slot(0, j)
            st_phase(1)
            for j in (3, 2, 1, 0):
                av_slot(1, j)

    nc.compile()
    return nc


def _masks():
    k = np.arange(128)[:, None]
    q = np.arange(128)[None, :]
    tril_t = np.where(k <= q, 0.0, NEG).astype(np.float32)  # S^T diag block
    fullneg = np.full((128, 128), NEG, np.float32)
    zeros = np.zeros((128, 128), np.float32)
    m_s0 = np.concatenate([tril_t, fullneg, zeros, tril_t], axis=1)
    m_s1 = np.concatenate([zeros, tril_t, tril_t, fullneg], axis=1)
    return m_s0, m_s1


def kernel(x, Wq, Wk, Wv):
    global LAST_EXEC_NS
    x = np.asarray(x, dtype=np.float32)
    Wq = np.asarray(Wq, dtype=np.float32)
    Wk = np.asarray(Wk, dtype=np.float32)
    Wv = np.asarray(Wv, dtype=np.float32)

    if "nc" not in _NC_CACHE:
        _NC_CACHE["nc"] = _build_nc()
    nc = _NC_CACHE["nc"]

    # host pre-transpose: x[b] (N, D) -> (tile, p=d%128, dchunk, token), bf16
    xt_all = np.ascontiguousarray(
        x.reshape(B, N_KTILES, 128, 8, 128).transpose(0, 1, 4, 3, 2).astype(BF)
    )  # [B, tile, p, c, q]

    # weights -> [p=d%128, dchunk, ecol], bf16
    wq_r = np.ascontiguousarray(Wq.reshape(8, 128, 1024).transpose(1, 0, 2).astype(BF))
    wk_r = np.ascontiguousarray(Wk.reshape(8, 128, 1024).transpose(1, 0, 2).astype(BF))
    wv_r = np.ascontiguousarray(Wv.reshape(8, 128, 1024).transpose(1, 0, 2).astype(BF))

    m_s0, m_s1 = _masks()
    in_maps = []
    for c in range(N_CORES):
        b, s = divmod(c, 2)
        in_maps.append({
            "x_t": xt_all[b],
            "x_qt": np.ascontiguousarray(xt_all[b, QSETS[s]]),
            "wq": wq_r, "wk": wk_r, "wv": wv_r,
            "mask": m_s1 if s else m_s0,
        })

    res = run_bass_kernel_spmd(nc, in_maps, list(range(N_CORES)), trace=TRACE)
    LAST_EXEC_NS = res.exec_time_ns

    out = np.empty((B, N, D), dtype=np.float32)
    for c in range(N_CORES):
        b, s = divmod(c, 2)
        oq = res.results[c]["out_q"]
        for j, g in enumerate(QSETS[s]):
            out[b, g * 128:(g + 1) * 128, :] = oq[j]
    return out
